# revision 19
# baseline (speedup 1.0000x reference)
"""Trainium2 Bass kernel for nn_ActorBatchNet (Set2Set + torsion MLP), v2.

Full inputs in, full output out. Data-parallel over graphs: 8 cores x 256
graphs. bf16 compute with fp32 accumulation.

Key structure vs v1:
- Set2Set attention (e scores / weighted r sums) on the Vector engine with
  graphs on partitions: elementwise mult + inner-axis tensor_reduce, instead
  of 1536 free-dim-2 PE matmuls.
- LSTM gates on PE in d-major layout (bf16), with per-step PE transposes to
  produce the row-major h and consume the row-major r.
- Torsion MLP: large batched indirect-DMA gathers of bf16 node rows, XBAR
  DMA-transpose to d-major, then 512-wide matmuls with W1 slot blocks as
  stationary weights. MLP overlaps Set2Set via interleaved emission.
"""

import sys

for _p in ("/opt/trn_rl_repo", "/root/.axon_site/_ro/trn_rl_repo"):
    if _p not in sys.path:
        sys.path.insert(0, _p)

import numpy as np
import ml_dtypes

import concourse.bass as bass
import concourse.bacc as bacc
import concourse.mybir as mybir
from concourse.tile import TileContext
from concourse.bass_utils import run_bass_kernel_spmd

F32 = mybir.dt.float32
BF16 = mybir.dt.bfloat16
I32 = mybir.dt.int32
AF = mybir.ActivationFunctionType
ALU = mybir.AluOpType
AX = mybir.AxisListType
BF16NP = ml_dtypes.bfloat16

# Problem constants
G = 2048
NPG = 64                 # nodes per graph
DIM = 128
TPG = 32                 # torsions per graph
ACTD = 36
STEPS = 6
NC = 8
NG = G // NC             # graphs per core = 256
NN = NG * NPG            # nodes per core = 16384
NT = NG * TPG            # torsions per core = 8192
P = 128
NHALF = 2                # graph halves (128 graphs each)
NCHUNK = 8               # MLP chunks
BPC = 8                  # 128-torsion tiles per chunk
TPC = BPC * P            # torsions per chunk = 1024

LAST = None
_CACHED = None


def build_bass():
    nc = bacc.Bacc("TRN2", target_bir_lowering=False, debug=False)

    # ---- DRAM parameters (per core) ----
    xgA = nc.declare_dram_parameter("xgA", [P, NHALF * NPG * DIM], BF16, isOutput=False)
    xfull = nc.declare_dram_parameter("xfull", [G * NPG, DIM], BF16, isOutput=False)
    idx = nc.declare_dram_parameter("idx", [P, NCHUNK * 4 * BPC], I32, isOutput=False)
    wAb = nc.declare_dram_parameter("wAb", [P, 4 * DIM], BF16, isOutput=False)
    wBb = nc.declare_dram_parameter("wBb", [P, 4 * DIM], BF16, isOutput=False)
    bs4 = nc.declare_dram_parameter("bs4", [P, 4], F32, isOutput=False)
    fcTq = nc.declare_dram_parameter("fcTq", [P, DIM], BF16, isOutput=False)
    fcTr = nc.declare_dram_parameter("fcTr", [P, DIM], BF16, isOutput=False)
    fcb = nc.declare_dram_parameter("fcb", [P, 1], F32, isOutput=False)
    w1geT = nc.declare_dram_parameter("w1geT", [P, DIM], BF16, isOutput=False)
    w1xT = nc.declare_dram_parameter("w1xT", [P, 4 * DIM], BF16, isOutput=False)
    b1 = nc.declare_dram_parameter("b1", [P, 1], F32, isOutput=False)
    w2Tb = nc.declare_dram_parameter("w2Tb", [P, ACTD], BF16, isOutput=False)
    b2r = nc.declare_dram_parameter("b2r", [P, ACTD], F32, isOutput=False)
    identb = nc.declare_dram_parameter("identb", [P, P], BF16, isOutput=False)
    out = nc.declare_dram_parameter("out", [NT, ACTD], F32, isOutput=True)

    import os
    DBG = os.environ.get("K_DEBUG_DUMP") == "1"
    if DBG:
        d_hrow = nc.declare_dram_parameter("d_hrow", [P, NHALF * DIM], BF16, isOutput=True)
        d_e = nc.declare_dram_parameter("d_e", [P, NHALF * NPG], BF16, isOutput=True)
        d_rrow = nc.declare_dram_parameter("d_rrow", [P, NHALF * DIM], BF16, isOutput=True)
        d_stage = nc.declare_dram_parameter("d_stage", [P, 4 * BPC * DIM], BF16, isOutput=True)
        d_zt = nc.declare_dram_parameter("d_zt", [P, 4 * BPC * DIM], BF16, isOutput=True)
        d_hdx = nc.declare_dram_parameter("d_hdx", [P, NT], BF16, isOutput=True)
        d_hgb = nc.declare_dram_parameter("d_hgb", [P, NG], BF16, isOutput=True)

    with nc.allow_low_precision("bf16 pipeline, tolerance 2e-2"), TileContext(nc) as tc:
        with tc.tile_pool(name="pc", bufs=1) as pc, \
             tc.tile_pool(name="pstage", bufs=3) as pstage, \
             tc.tile_pool(name="pprod", bufs=1) as pprod, \
             tc.tile_pool(name="plsb", bufs=2) as plsb, \
             tc.tile_pool(name="pg", bufs=2, space="PSUM") as pg, \
             tc.tile_pool(name="phd", bufs=2, space="PSUM") as phd, \
             tc.tile_pool(name="ptr", bufs=2, space="PSUM") as ptr, \
             tc.tile_pool(name="plg", bufs=2, space="PSUM") as plg:

            # ---- persistent SBUF ----
            xgA_sb = pc.tile([P, NHALF * NPG * DIM], BF16, tag="xgA")
            zT_tiles = []
            for _c in range(NCHUNK):
                zt_c = pc.tile([P, 4 * BPC * P], BF16, tag=f"zT{_c}")
                zT_tiles.append(zt_c)
            idx_sb = pc.tile([P, NCHUNK * 4 * BPC], I32, tag="idx")
            wA_sb = pc.tile([P, 4 * DIM], BF16, tag="wA")
            wB_sb = pc.tile([P, 4 * DIM], BF16, tag="wB")
            bs_sb = pc.tile([P, 4], F32, tag="bs")
            fcq_sb = pc.tile([P, DIM], BF16, tag="fcq")
            fcr_sb = pc.tile([P, DIM], BF16, tag="fcr")
            fcb_sb = pc.tile([P, 1], F32, tag="fcb")
            w1g_sb = pc.tile([P, DIM], BF16, tag="w1g")
            w1x_sb = pc.tile([P, 4 * DIM], BF16, tag="w1x")
            b1_sb = pc.tile([P, 1], F32, tag="b1")
            w2_sb = pc.tile([P, ACTD], BF16, tag="w2")
            b2_sb = pc.tile([P, ACTD], F32, tag="b2")
            id_sb = pc.tile([P, P], BF16, tag="id")

            hTb = pc.tile([P, NG], BF16, tag="hTb")       # [d, g] bf16
            rTb = pc.tile([P, NG], BF16, tag="rTb")       # [d, g] bf16
            cT = pc.tile([P, NG], F32, tag="cT")          # [d, g] f32
            hrow = pc.tile([P, NHALF * DIM], BF16, tag="hrow")   # [g, (half, d)]
            iS = pc.tile([P, NG], F32, tag="iS")
            fS = pc.tile([P, NG], F32, tag="fS")
            oS = pc.tile([P, NG], F32, tag="oS")
            gT = pc.tile([P, NG], F32, tag="gT")
            tnc = pc.tile([P, NG], F32, tag="tnc")
            e_sb = pc.tile([P, NHALF * NPG], BF16, tag="e")       # [g, (half, n)]
            expe = pc.tile([P, NHALF * NPG], BF16, tag="expe")
            expeX = pc.tile([P, NHALF * NPG * DIM], BF16, tag="expeX")
            sumex = pc.tile([P, NHALF], F32, tag="sumex")
            recip = pc.tile([P, NHALF], F32, tag="recip")
            runm = pc.tile([P, NHALF * DIM], BF16, tag="runm")    # [g, (half, d)]
            rrow = pc.tile([P, NHALF * DIM], BF16, tag="rrow")
            geb = pc.tile([P, NG], BF16, tag="geb")
            hdx = pc.tile([P, NT], BF16, tag="hdx")       # [h, t]

            # ---- input loads (sync engine) ----
            nc.sync.dma_start(out=idx_sb[:], in_=idx[:, :])
            nc.sync.dma_start(out=wA_sb[:], in_=wAb[:, :])
            nc.sync.dma_start(out=wB_sb[:], in_=wBb[:, :])
            nc.sync.dma_start(out=bs_sb[:], in_=bs4[:, :])
            nc.sync.dma_start(out=id_sb[:], in_=identb[:, :])
            nc.sync.dma_start(out=fcq_sb[:], in_=fcTq[:, :])
            nc.sync.dma_start(out=fcr_sb[:], in_=fcTr[:, :])
            nc.sync.dma_start(out=fcb_sb[:], in_=fcb[:, :])
            nc.sync.dma_start(out=w1g_sb[:], in_=w1geT[:, :])
            nc.sync.dma_start(out=w1x_sb[:], in_=w1xT[:, :])
            nc.sync.dma_start(out=b1_sb[:], in_=b1[:, :])
            nc.sync.dma_start(out=w2_sb[:], in_=w2Tb[:, :])
            nc.sync.dma_start(out=b2_sb[:], in_=b2r[:, :])
            HB = NPG * DIM  # 8192 elems per half
            for h in range(NHALF):
                nc.sync.dma_start(out=xgA_sb[:, h * HB:(h + 1) * HB],
                                  in_=xgA[:, h * HB:(h + 1) * HB])

            nc.vector.memset(hTb[:], 0.0)
            nc.vector.memset(rTb[:], 0.0)
            nc.vector.memset(cT[:], 0.0)

            # ---- MLP gathers (gpsimd; all emitted up front, throttled by
            # the staging pool) and chunk state ----
            stage_tiles = []
            for c in range(NCHUNK):
                st = pstage.tile([P, 4 * BPC * DIM], BF16, tag="stage")
                # multi-column offset APs are broken on HW: one indirect DMA
                # per 128 rows (offset AP [128, 1])
                for j in range(32):
                    nc.gpsimd.indirect_dma_start(
                        out=st[:, j * DIM:(j + 1) * DIM],
                        out_offset=None,
                        in_=xfull[:, :],
                        in_offset=bass.IndirectOffsetOnAxis(
                            ap=idx_sb[:, c * 32 + j:c * 32 + j + 1], axis=0),
                    )
                stage_tiles.append(st)

            def bcast(ap, ins_pos, n):
                """Insert a stride-0 dim of size n at free position ins_pos."""
                l = list(ap.ap)
                l.insert(ins_pos, [0, n])
                return bass.AP(ap.tensor, ap.offset, l)

            CW = 4 * BPC * DIM  # cols per chunk in zT

            def emit_transpose_chunk(c):
                st = stage_tiles[c]
                # per-128-column-group transpose: zt[d, j, t] = st[t, j, d]
                nc.sync.dma_start_transpose(
                    out=zT_tiles[c][:].rearrange("p (j t) -> p j t", j=4 * BPC),
                    in_=st[:])
                if DBG and c == 0:
                    nc.sync.dma_start(out=d_stage[:, :], in_=st[:])
                    nc.sync.dma_start(out=d_zt[:, :], in_=zT_tiles[0][:])

            def emit_mlp_chunk_mm(c):
                zt3 = zT_tiles[c][:].rearrange(
                    "p (s b t) -> p s b t", s=4, b=BPC)
                for sub in range(2):
                    ph = phd.tile([P, 512], F32, tag="hd")
                    # per-graph term W1_ge @ ge, broadcast over the 32
                    # torsions of each graph via a stride-0 rhs dim. Depends
                    # on geb, which forces ALL MLP psum work after set2set.
                    g0 = (c * TPC + sub * 512) // TPG
                    gsl = geb[:, g0:g0 + 16]
                    nc.tensor.matmul(
                        out=ph[:], lhsT=w1g_sb[:],
                        rhs=bcast(gsl, 2, TPG),
                        start=True, stop=False)
                    for s in range(4):
                        nc.tensor.matmul(
                            out=ph[:],
                            lhsT=w1x_sb[:, s * DIM:(s + 1) * DIM],
                            rhs=zt3[:, s, sub * 4:(sub + 1) * 4, :],
                            start=False, stop=(s == 3))
                    nc.vector.tensor_scalar(
                        out=hdx[:, c * TPC + sub * 512: c * TPC + (sub + 1) * 512],
                        in0=ph[:], scalar1=b1_sb[:, 0:1], scalar2=0.0,
                        op0=ALU.add, op1=ALU.max)


            # ---- Set2Set (6 steps); chunk transposes interleaved on the
            # scalar queue (one-ish per step, after the step's ACT work) ----
            tnext = 0
            for step in range(STEPS):
                # gates (PE): psum tiles [128, 512] f32, 2 gates per tile
                gpa = pg.tile([P, 512], F32, tag="gates")
                gpb = pg.tile([P, 512], F32, tag="gates")
                gslice = [gpa[:, 0:NG], gpa[:, NG:2 * NG],
                          gpb[:, 0:NG], gpb[:, NG:2 * NG]]
                for k in range(4):
                    nc.tensor.matmul(out=gslice[k],
                                     lhsT=wA_sb[:, k * P:(k + 1) * P],
                                     rhs=hTb[:], start=True, stop=False)
                    nc.tensor.matmul(out=gslice[k],
                                     lhsT=wB_sb[:, k * P:(k + 1) * P],
                                     rhs=rTb[:], start=False, stop=True)
                # LSTM pointwise (i, f, g, o)
                nc.scalar.activation(out=iS[:], in_=gslice[0], func=AF.Sigmoid,
                                     bias=bs_sb[:, 0:1])
                nc.scalar.activation(out=fS[:], in_=gslice[1], func=AF.Sigmoid,
                                     bias=bs_sb[:, 1:2])
                nc.scalar.activation(out=gT[:], in_=gslice[2], func=AF.Tanh,
                                     bias=bs_sb[:, 2:3])
                nc.scalar.activation(out=oS[:], in_=gslice[3], func=AF.Sigmoid,
                                     bias=bs_sb[:, 3:4])
                nc.vector.tensor_mul(out=cT[:], in0=fS[:], in1=cT[:])
                nc.vector.tensor_mul(out=iS[:], in0=iS[:], in1=gT[:])
                nc.vector.tensor_add(out=cT[:], in0=cT[:], in1=iS[:])
                nc.scalar.activation(out=tnc[:], in_=cT[:], func=AF.Tanh)
                nc.vector.tensor_mul(out=hTb[:], in0=oS[:], in1=tnc[:])

                # transpose h to row-major [g, d] per half
                for h in range(NHALF):
                    tp = ptr.tile([P, P], BF16, tag="tr")
                    nc.tensor.transpose(out=tp[:], in_=hTb[:, h * P:(h + 1) * P],
                                        identity=id_sb[:])
                    nc.vector.tensor_copy(out=hrow[:, h * DIM:(h + 1) * DIM], in_=tp[:])

                # attention on DVE with tree reductions (TT runs at the 2x
                # bf16 rate on HW, TensorReduce does not). The softmax weights
                # are expanded on the Scalar engine so the r-mult's in1 has a
                # stride-1 innermost dim (keeps DVE in 2x mode); halves are
                # pipelined so Scalar expansion overlaps DVE work.
                for h in range(NHALF):
                    xsl = xgA_sb[:, h * HB:(h + 1) * HB].rearrange(
                        "p (n d) -> p n d", d=DIM)
                    prodE = pprod.tile([P, NPG * DIM], BF16, tag="prod")
                    pE3 = prodE[:].rearrange("p (n d) -> p n d", d=DIM)
                    hsl = hrow[:, h * DIM:(h + 1) * DIM]
                    nc.vector.tensor_mul(out=pE3, in0=xsl, in1=bcast(hsl, 1, NPG))
                    # e: reduce over innermost d: 2 tree levels + reduce
                    nc.vector.tensor_add(
                        out=pE3[:, :, 0:64], in0=pE3[:, :, 0:64], in1=pE3[:, :, 64:128])
                    nc.vector.tensor_add(
                        out=pE3[:, :, 0:32], in0=pE3[:, :, 0:32], in1=pE3[:, :, 32:64])
                    nc.vector.tensor_reduce(
                        out=e_sb[:, h * NPG:(h + 1) * NPG], in_=pE3[:, :, 0:32],
                        axis=AX.X, op=ALU.add)
                    nc.scalar.activation(
                        out=expe[:, h * NPG:(h + 1) * NPG],
                        in_=e_sb[:, h * NPG:(h + 1) * NPG],
                        func=AF.Exp, accum_out=sumex[:, h:h + 1])
                    esl = expe[:, h * NPG:(h + 1) * NPG]
                    nc.scalar.activation(
                        out=expeX[:, h * HB:(h + 1) * HB].rearrange(
                            "p (n d) -> p n d", d=DIM),
                        in_=bcast(esl, 2, DIM), func=AF.Copy)
                for h in range(NHALF):
                    xsl = xgA_sb[:, h * HB:(h + 1) * HB].rearrange(
                        "p (n d) -> p n d", d=DIM)
                    prodR = pprod.tile([P, NPG * DIM], BF16, tag="prod")
                    pR3 = prodR[:].rearrange("p (n d) -> p n d", d=DIM)
                    nc.vector.tensor_mul(
                        out=pR3, in0=xsl,
                        in1=expeX[:, h * HB:(h + 1) * HB].rearrange(
                            "p (n d) -> p n d", d=DIM))
                    n2 = NPG
                    while n2 > 1:
                        n2 //= 2
                        nc.vector.tensor_add(
                            out=pR3[:, 0:n2, :], in0=pR3[:, 0:n2, :],
                            in1=pR3[:, n2:2 * n2, :])
                    nc.vector.tensor_copy(
                        out=runm[:, h * DIM:(h + 1) * DIM], in_=pR3[:, 0, :])
                nc.vector.reciprocal(out=recip[:], in_=sumex[:])
                for h in range(NHALF):
                    nc.vector.tensor_scalar_mul(
                        out=rrow[:, h * DIM:(h + 1) * DIM],
                        in0=runm[:, h * DIM:(h + 1) * DIM],
                        scalar1=recip[:, h:h + 1])
                    tp = ptr.tile([P, P], BF16, tag="tr")
                    nc.tensor.transpose(out=tp[:], in_=rrow[:, h * DIM:(h + 1) * DIM],
                                        identity=id_sb[:])
                    nc.vector.tensor_copy(out=rTb[:, h * P:(h + 1) * P], in_=tp[:])
                want = step * NCHUNK // STEPS if step < STEPS - 1 else NCHUNK
                while tnext < want:
                    emit_transpose_chunk(tnext)
                    tnext += 1

                if DBG and step == 0:
                    nc.sync.dma_start(out=d_hrow[:, :], in_=hrow[:])
                    nc.sync.dma_start(out=d_e[:, :], in_=e_sb[:])
                    nc.sync.dma_start(out=d_rrow[:, :], in_=rrow[:])


            # ---- graph embed + per-graph MLP hidden term ----
            gep = pg.tile([P, 512], F32, tag="gates")
            nc.tensor.matmul(out=gep[:, 0:NG], lhsT=fcq_sb[:], rhs=hTb[:],
                             start=True, stop=False)
            nc.tensor.matmul(out=gep[:, 0:NG], lhsT=fcr_sb[:], rhs=rTb[:],
                             start=False, stop=True)
            nc.scalar.activation(out=geb[:], in_=gep[:, 0:NG], func=AF.Identity,
                                 bias=fcb_sb[:, 0:1])

            # ---- MLP matmuls (PE) after set2set so PE never stalls on the
            # gather during the steps ----
            for c in range(NCHUNK):
                emit_mlp_chunk_mm(c)

            if DBG:
                nc.sync.dma_start(out=d_hdx[:, :], in_=hdx[:])

            for c2 in range(NCHUNK):
                lp = plg.tile([P, BPC * ACTD], F32, tag="lg")
                lp3 = lp[:].rearrange("p (b a) -> p b a", a=ACTD)
                for b in range(BPC):
                    t0 = (c2 * BPC + b) * P
                    nc.tensor.matmul(out=lp3[:, b, :],
                                     lhsT=hdx[:, t0:t0 + P],
                                     rhs=w2_sb[:], start=True, stop=True)
                lsb = plsb.tile([P, BPC * ACTD], F32, tag="lsb")
                nc.vector.tensor_add(
                    out=lsb[:].rearrange("p (b a) -> p b a", a=ACTD),
                    in0=lp3, in1=bcast(b2_sb[:], 1, BPC))
                nc.sync.dma_start(
                    out=out[c2 * TPC:(c2 + 1) * TPC, :].rearrange(
                        "(b p) a -> p b a", p=P),
                    in_=lsb[:].rearrange("p (b a) -> p b a", a=ACTD))
    nc.compile()
    return nc


def _host_prep(inputs):
    x = np.asarray(inputs["x"], np.float32)
    nonring = np.asarray(inputs["nonring"], np.int32)
    w_ih = np.asarray(inputs["w_ih"], np.float32)
    w_hh = np.asarray(inputs["w_hh"], np.float32)
    b_ih = np.asarray(inputs["b_ih"], np.float32)
    b_hh = np.asarray(inputs["b_hh"], np.float32)
    fc_w = np.asarray(inputs["fc_w"], np.float32)
    fc_b = np.asarray(inputs["fc_b"], np.float32)
    mlp_w1 = np.asarray(inputs["mlp_w1"], np.float32)
    mlp_b1 = np.asarray(inputs["mlp_b1"], np.float32)
    mlp_w2 = np.asarray(inputs["mlp_w2"], np.float32)
    mlp_b2 = np.asarray(inputs["mlp_b2"], np.float32)

    bf = lambda a: np.ascontiguousarray(a).astype(BF16NP)
    fcT = fc_w.T                       # [256, 128]
    w1T = mlp_w1.T                     # [640, 128]
    # w1xT packed [128, 4, 128]: w1xT[p, s, h] = w1T[128 + s*128 + p, h]
    w1x = np.ascontiguousarray(
        w1T[DIM:].reshape(4, DIM, DIM).transpose(1, 0, 2).reshape(DIM, 4 * DIM))
    rep = {
        "xfull": bf(x),
        "wAb": bf((w_ih[:, :DIM] + w_hh).T),
        "wBb": bf(w_ih[:, DIM:].T),
        "bs4": np.ascontiguousarray((b_ih + b_hh).reshape(4, P).T),
        "fcTq": bf(fcT[:DIM]),
        "fcTr": bf(fcT[DIM:]),
        "fcb": np.ascontiguousarray(fc_b.reshape(P, 1)),
        "w1geT": bf(w1T[:DIM]),
        "w1xT": bf(w1x),
        "b1": np.ascontiguousarray(mlp_b1.reshape(P, 1)),
        "w2Tb": bf(mlp_w2.T),
        "b2r": np.ascontiguousarray(np.tile(mlp_b2.reshape(1, ACTD), (P, 1))),
        "identb": np.eye(P, dtype=np.float32).astype(BF16NP),
    }

    in_maps = []
    for k in range(NC):
        xc = x[k * NN:(k + 1) * NN].reshape(NG, NPG, DIM)
        xch = xc.reshape(NHALF, P, NPG, DIM)
        xgA = bf(xch.transpose(1, 0, 2, 3).reshape(P, NHALF * NPG * DIM))
        nr = nonring[k * NT:(k + 1) * NT]                    # [8192, 4]
        # idx[p, c, s, b] = nr[(c*BPC + b)*128 + p, s]
        idxp = nr.reshape(NCHUNK, BPC, P, 4).transpose(2, 0, 3, 1)
        m = dict(rep)
        m["xgA"] = xgA
        m["idx"] = np.ascontiguousarray(idxp.reshape(P, NCHUNK * 4 * BPC)).astype(np.int32)
        in_maps.append(m)
    return in_maps


def kernel(**inputs) -> np.ndarray:
    global LAST, _CACHED
    if _CACHED is None:
        _CACHED = build_bass()
    nc = _CACHED
    in_maps = _host_prep(inputs)
    LAST = run_bass_kernel_spmd(nc, in_maps, core_ids=list(range(NC)))
    outs = [np.asarray(LAST.results[k]["out"], np.float32).reshape(NG, TPG, ACTD)
            for k in range(NC)]
    return np.concatenate(outs, axis=0)


if __name__ == "__main__":
    nc = build_bass()
    print("build ok")
